# revision 1
# baseline (speedup 1.0000x reference)
"""Trainium2 Bass kernel for CrossAttention (self-attention variant).

Reference computation (fp32):
    q = x @ Wq.T ; k = x @ Wk.T ; v = x @ Wv.T     (B,N,D) @ (D,D)
    per head (16 heads, dh=64): s = q k^T * dh^-0.5 ; p = softmax(s)
    o = p v ; out = concat(o) @ Wout.T + bout

Sharding: batch*heads across 8 cores. Core c handles batch c//4 and the
4 heads 4*(c%4)..4*(c%4)+3 (a contiguous 256-wide slice of the inner dim).
Each core computes its partial out = o_slice @ Wout[:, slice].T ; the host
sums the 4 partials per batch and adds the bias.

On-device layout trick: everything the PE contracts over must sit on the
SBUF partition axis, so the host ships x and the weight slices already
transposed (xT = x[b].T etc.).  Attention is computed in the transposed
layout sT[j, i] = k_j . q_i so no on-device transposes are needed at all:
  - s-matmul: lhsT = kT[dh, j-tile], rhs = qT[dh, i-block]
  - p = exp(s * scale)  (softmax max-subtraction skipped: |s*scale| < ~3)
  - o-matmul: lhsT = v_aug[j, 65] (col 64 = ones), rhs = p[j, i-block]
    -> oT[d, i] with the softmax denominator in row 64.
  - normalization: recip of the denominator row is broadcast across
    partitions with a K=1 matmul, then fused into the PSUM->SBUF copy.
  - out-projection: lhsT = oT (already transposed!), rhs = WoutT.
"""

import numpy as np

B, N, D = 2, 2048, 1024
H, DH = 16, 64
SCALE = DH**-0.5
NCORES = 8
HLOC = H // 4  # 4 heads per core
DLOC = HLOC * DH  # 256-wide inner slice per core
P = 128

# matmul operand dtype: "f32" (exact, slow), "f32r" (fp32 replicated, fast),
# "bf16" (fast, lower precision)
MM_MODE = "f32r"

_cached = {}


def _build(mm_mode=MM_MODE, repeat=1):
    import concourse.bass as bass
    import concourse.tile as tile
    from concourse import bacc, mybir

    mm_mode, *variants = mm_mode.split("+")
    variants = set(variants)

    f32 = mybir.dt.float32
    Exp = mybir.ActivationFunctionType.Exp

    if mm_mode == "bf16":
        io_dt = mybir.dt.bfloat16
    elif mm_mode == "f32r":
        # fp32r matmul operands must be *produced* as fp32r (the BIR
        # verifier requires rounding at the producer), so the whole
        # activation/weight path is typed float32r; PSUM stays fp32.
        io_dt = mybir.dt.float32r
    else:
        io_dt = f32

    def mm_ap(ap):
        return ap

    nc = bacc.Bacc("TRN2", target_bir_lowering=False, debug=False)

    xT = nc.dram_tensor("xT", [D, N], io_dt, kind="ExternalInput").ap()
    wqT = nc.dram_tensor("wqT", [D, DLOC], io_dt, kind="ExternalInput").ap()
    wkT = nc.dram_tensor("wkT", [D, DLOC], io_dt, kind="ExternalInput").ap()
    wvT = nc.dram_tensor("wvT", [D, DLOC], io_dt, kind="ExternalInput").ap()
    woutT = nc.dram_tensor("woutT", [DLOC, D], io_dt, kind="ExternalInput").ap()
    out = nc.dram_tensor("out", [N, D], f32, kind="ExternalOutput").ap()

    CT = D // P  # 8 c-tiles (contraction tiles for projections)
    NT = N // P  # 16 seq tiles
    DT2 = DLOC // P  # 2 local d-tiles

    with tile.TileContext(nc) as tc:
        s_bufs, o_bufs = (3, 1) if "s3o2" in variants else (2, 2)
        stage_bufs = 3 if "p3" in variants else 2
        with (
            tc.tile_pool(name="big", bufs=1) as big,
            tc.tile_pool(name="stage", bufs=stage_bufs) as stage,
            tc.tile_pool(name="small", bufs=1) as small,
            tc.tile_pool(name="ps_s", bufs=s_bufs, space="PSUM") as ps_s,
            tc.tile_pool(name="ps_o", bufs=o_bufs, space="PSUM") as ps_o,
        ):
            ones_sb = small.tile([1, P], f32, tag="ones")
            nc.vector.memset(ones_sb[:], 1.0)

            for rep in range(repeat):
                _emit_iter(
                    nc, tile, mybir, f32, Exp, io_dt, mm_ap, rep, variants,
                    big, stage, small, ps_s, ps_o, ones_sb,
                    xT, wqT, wkT, wvT, woutT, out,
                    CT, NT, DT2,
                )

    nc.compile()
    return nc


def _emit_iter(
    nc, tile, mybir, f32, Exp, io_dt, mm_ap, rep, variants,
    big, stage, small, ps_s, ps_o, ones_sb,
    xT, wqT, wkT, wvT, woutT, out,
    CT, NT, DT2,
):
    # ---- resident SBUF tensors ------------------------------------
    xT_sb = big.tile([P, CT, N], io_dt, tag="xT", name=f"xT_sb_{rep}")
    wqT_sb = big.tile([P, CT, DLOC], io_dt, tag="wqT", name=f"wqT_sb_{rep}")
    wkT_sb = big.tile([P, CT, DLOC], io_dt, tag="wkT", name=f"wkT_sb_{rep}")
    wvT_sb = big.tile([P, CT, DLOC], io_dt, tag="wvT", name=f"wvT_sb_{rep}")
    woutT_sb = big.tile([P, DT2, D], io_dt, tag="woutT", name=f"woutT_sb_{rep}")
    qT_sb = big.tile([P, DT2, N], io_dt, tag="qT", name=f"qT_sb_{rep}")
    kT_sb = big.tile([P, DT2, N], io_dt, tag="kT", name=f"kT_sb_{rep}")
    v_sb = big.tile([P, NT, HLOC * (DH + 1)], io_dt, tag="v", name=f"v_sb_{rep}")
    oT_sb = big.tile([P, DT2, N], io_dt, tag="oT", name=f"oT_sb_{rep}")

    for h in range(HLOC):
        # the softmax-denominator ones column of v_aug. memset can't emit
        # float32r, so write the fp32 bit pattern of 1.0 through uint32.
        col = v_sb[:, :, h * (DH + 1) + DH]
        if io_dt == mybir.dt.float32r:
            nc.vector._memset_packed(col.bitcast(mybir.dt.uint32), 0x3F800000)
        else:
            nc.vector.memset(col, 1.0)

    # ---- input DMAs (weights first; xT split per c-tile so the
    # projection accumulation overlaps the load) ---------------------
    nc.sync.dma_start(wqT_sb[:], wqT.rearrange("(c p) d -> p c d", p=P))
    nc.sync.dma_start(wkT_sb[:], wkT.rearrange("(c p) d -> p c d", p=P))
    nc.sync.dma_start(wvT_sb[:], wvT.rearrange("(c p) d -> p c d", p=P))
    nc.sync.dma_start(woutT_sb[:], woutT.rearrange("(t p) d -> p t d", p=P))
    for ct in range(CT):
        nc.sync.dma_start(xT_sb[:, ct, :], xT[ct * P : ct * P + P, :])

    # ---- projections ----------------------------------------------
    def proj_qk(w_sb, dst, dt_, ih):
        ps = ps_s.tile([P, 1024], f32, tag="s", name=f"psqk_{rep}_{id(w_sb)}_{dt_}_{ih}")
        for ct in range(CT):
            for half in range(2):
                nc.tensor.matmul(
                    ps[:, half * 512 : half * 512 + 512],
                    mm_ap(w_sb[:, ct, dt_ * P : dt_ * P + P]),
                    mm_ap(
                        xT_sb[
                            :, ct,
                            ih * 1024 + half * 512 : ih * 1024 + half * 512 + 512,
                        ]
                    ),
                    start=(ct == 0),
                    stop=(ct == CT - 1),
                )
        nc.vector.tensor_copy(dst[:, dt_, ih * 1024 : ih * 1024 + 1024], ps[:])

    # qT[d, i] = sum_c WqT[c, d] xT[c, i]; attention on i-block 0 needs
    # ih=0 of every head plus all of v, so emit in that order.
    for dt_ in range(DT2):
        proj_qk(wqT_sb, qT_sb, dt_, 0)
        proj_qk(wkT_sb, kT_sb, dt_, 0)

    # v[j, d] = sum_c xT[c, j] WvT[c, d]  (natural layout, + ones col)
    for jt in range(NT):
        psv = ps_s.tile([P, 1024], f32, tag="s", name=f"psv_{rep}_{jt}")
        for ct in range(CT):
            nc.tensor.matmul(
                psv[:, :DLOC],
                mm_ap(xT_sb[:, ct, jt * P : jt * P + P]),
                mm_ap(wvT_sb[:, ct, :]),
                start=(ct == 0),
                stop=(ct == CT - 1),
            )
        # one strided copy fans the 4 heads out into the 65-wide slots
        nc.vector.tensor_copy(
            v_sb[:, jt, :].rearrange("p (h u) -> p h u", u=DH + 1)[:, :, :DH],
            psv[:, :DLOC].rearrange("p (h u) -> p h u", u=DH),
        )

    for dt_ in range(DT2):
        proj_qk(wqT_sb, qT_sb, dt_, 1)
        proj_qk(wkT_sb, kT_sb, dt_, 1)

    # ---- attention + output projection, one 1024-wide i-block at a
    # time so the out-projection and its DMA overlap the next block ---
    den_sb = small.tile([1, HLOC, 1024], f32, tag="den", name=f"den_{rep}")
    for ib2 in range(2):
        i0 = ib2 * 1024
        for h in range(HLOC):
            hp = h // 2  # which 128-partition block of qT/kT
            ho = (h % 2) * DH  # partition offset within the block
            po = ps_o.tile([DH + 1, 1024], f32, tag="o", name=f"po_{rep}_{ib2}_{h}")
            for jt in range(NT):
                pss = ps_s.tile([P, 1024], f32, tag="s", name=f"pss_{rep}_{ib2}_{h}_{jt}")
                for half in range(2):
                    nc.tensor.matmul(
                        pss[:, half * 512 : half * 512 + 512],
                        mm_ap(kT_sb[ho : ho + DH, hp, jt * P : jt * P + P]),
                        mm_ap(
                            qT_sb[
                                ho : ho + DH, hp,
                                i0 + half * 512 : i0 + half * 512 + 512,
                            ]
                        ),
                        start=True,
                        stop=True,
                    )
                p_sb = stage.tile(
                    [P, 1024], io_dt, tag="p", name=f"p_sb_{rep}_{ib2}_{h}_{jt}"
                )
                if "noexp" in variants:  # timing diagnostic: DVE instead of ACT
                    nc.vector.tensor_copy(p_sb[:], pss[:])
                else:
                    nc.scalar.activation(p_sb[:], pss[:], Exp, scale=SCALE)
                for half in range(2):
                    nc.tensor.matmul(
                        po[:, half * 512 : half * 512 + 512],
                        mm_ap(v_sb[:, jt, h * (DH + 1) : (h + 1) * (DH + 1)]),
                        mm_ap(p_sb[:, half * 512 : half * 512 + 512]),
                        start=(jt == 0),
                        stop=(jt == NT - 1),
                    )
            # drain PSUM immediately: unnormalized oT + denominator row.
            # Normalization itself is deferred and batched below so the
            # recip/broadcast chain never stalls the next head's matmuls.
            nc.vector.tensor_copy(
                oT_sb[ho : ho + DH, hp, i0 : i0 + 1024], po[:DH, :]
            )
            nc.vector.tensor_copy(den_sb[:, h, :], po[DH : DH + 1, :])

        # batched normalization: oT[d, i] /= den[i] per head
        if "nonorm" not in variants:
            for h in range(HLOC):
                hp = h // 2
                ho = (h % 2) * DH
                recip = small.tile(
                    [1, 1024], f32, tag="recip", name=f"recip_{rep}_{ib2}_{h}"
                )
                nc.vector.reciprocal(recip[:], den_sb[:, h, :])
                bc = ps_o.tile([DH + 1, 1024], f32, tag="o", name=f"bc_{rep}_{ib2}_{h}")
                for half in range(2):
                    nc.tensor.matmul(
                        bc[:DH, half * 512 : half * 512 + 512],
                        ones_sb[:, :DH],
                        recip[:, half * 512 : half * 512 + 512],
                        start=True,
                        stop=True,
                    )
                dst = oT_sb[ho : ho + DH, hp, i0 : i0 + 1024]
                nc.vector.tensor_mul(dst, dst, bc[:DH, :])

        # out[i, do] = sum_d oT[d, i] WoutT[d, do] for this i-block
        for it in range(ib2 * 8, ib2 * 8 + 8):
            po = ps_s.tile([P, 1024], f32, tag="s", name=f"pso_{rep}_{it}")
            for db in range(2):
                for dt_ in range(DT2):
                    nc.tensor.matmul(
                        po[:, db * 512 : db * 512 + 512],
                        mm_ap(oT_sb[:, dt_, it * P : it * P + P]),
                        mm_ap(woutT_sb[:, dt_, db * 512 : db * 512 + 512]),
                        start=(dt_ == 0),
                        stop=(dt_ == DT2 - 1),
                    )
            ob = stage.tile([P, 1024], f32, tag="ob", name=f"ob_{rep}_{it}")
            nc.vector.tensor_copy(ob[:], po[:])
            nc.sync.dma_start(out[it * P : it * P + P, :], ob[:])


def get_nc(mm_mode=MM_MODE, repeat=1):
    key = (mm_mode, repeat)
    if key not in _cached:
        _cached[key] = _build(mm_mode, repeat)
    return _cached[key]


def make_in_maps(x, Wq, Wk, Wv, Wout, mm_mode=MM_MODE):
    if mm_mode == "bf16":
        import ml_dtypes

        cast = lambda a: np.ascontiguousarray(np.asarray(a), dtype=ml_dtypes.bfloat16)
    else:
        cast = lambda a: np.ascontiguousarray(np.asarray(a), dtype=np.float32)
    x, Wq, Wk, Wv, Wout = (np.asarray(a) for a in (x, Wq, Wk, Wv, Wout))
    in_maps = []
    for c in range(NCORES):
        b = c // 4
        rows = slice((c % 4) * DLOC, (c % 4 + 1) * DLOC)
        in_maps.append(
            {
                "xT": cast(x[b].T),
                "wqT": cast(Wq[rows].T),
                "wkT": cast(Wk[rows].T),
                "wvT": cast(Wv[rows].T),
                "woutT": cast(Wout[:, rows].T),
            }
        )
    return in_maps


def kernel(x, Wq, Wk, Wv, Wout, bout):
    from concourse.bass_utils import run_bass_kernel_spmd

    nc = get_nc()
    in_maps = make_in_maps(x, Wq, Wk, Wv, Wout)
    res = run_bass_kernel_spmd(nc, in_maps, list(range(NCORES)))
    out = np.zeros((B, N, D), np.float32)
    for c in range(NCORES):
        out[c // 4] += res.results[c]["out"]
    out += np.asarray(bout, np.float32)
    return out



# revision 15
# speedup vs baseline: 1.7928x; 1.7928x over previous
"""Trainium2 Bass kernel for CrossAttention (self-attention variant).

Reference computation (fp32):
    q = x @ Wq.T ; k = x @ Wk.T ; v = x @ Wv.T     (B,N,D) @ (D,D)
    per head (16 heads, dh=64): s = q k^T * dh^-0.5 ; p = softmax(s)
    o = p v ; out = concat(o) @ Wout.T + bout

Sharding: batch*heads across 8 cores. Core c handles batch c//4 and the
4 heads 4*(c%4)..4*(c%4)+3 (a contiguous 256-wide slice of the inner dim).
Each core computes its partial out = o_slice @ Wout[:, slice].T ; the host
sums the 4 partials per batch and adds the bias.

Performance design (vs the v1 kernel):
  - all matmul operands in bf16 (1 cycle/row on the PE, same as fp32r at
    512-wide moving, but half the DMA bytes and SBUF footprint).
  - The attention inner loop is ACT(exp)-bound: 128 exp instructions over
    [128, 1024] tiles ~= 135us.  Everything else is scheduled around that
    spine:
      * s-matmuls for the two heads sharing a 128-partition block (dh=64
        each at partitions 0-63 / 64-127) are emitted back-to-back; the PE
        runs them CONCURRENTLY on separate row-groups (tile_position row
        tiling), so a 512-i-block s-pair costs ~512 cycles, not 1024.
      * one exp covers both heads' 512-wide s halves.
      * per-head o accumulators are [65, 512] (one PSUM bank each), with
        the softmax denominator in row 64 via the ones-column of v_aug.
      * the remaining q-projection chunks, the softmax normalization
        (reciprocal + K=1 broadcast matmul + multiply) and the output
        projection are injected as "filler" PE work into the spine's
        slack, at most ~one 512-col matmul per exp slot, so the ACT
        stream never starves and the PE never idles.
  - DMA issue order follows need order: wk, first xT chunks, wq, rest of
    xT, wv, wout.  The k/v projections consume xT chunk-by-chunk as the
    loads land.
"""

import numpy as np

B, N, D = 2, 2048, 1024
H, DH = 16, 64
SCALE = DH**-0.5
NCORES = 8
HLOC = H // 4  # 4 heads per core
DLOC = HLOC * DH  # 256-wide inner slice per core
P = 128
IB = 512  # attention i-block
NIB = N // IB  # 4

MM_MODE = "bf16"

_cached = {}


def _build(mm_mode=MM_MODE, repeat=1):
    import concourse.bass as bass
    import concourse.tile as tile
    from concourse import bacc, mybir

    mm_mode, *variants = mm_mode.split("+")
    variants = set(variants)

    f32 = mybir.dt.float32
    f32r = mybir.dt.float32r

    if mm_mode == "bf16":
        io_dt = mybir.dt.bfloat16
    elif mm_mode == "f32r":
        io_dt = f32r
    else:
        io_dt = f32

    nc = bacc.Bacc("TRN2", target_bir_lowering=False, debug=False)

    dbg = "dbg" in variants
    xT = nc.dram_tensor("xT", [D, N], io_dt, kind="ExternalInput").ap()
    wqT = nc.dram_tensor("wqT", [D, DLOC], io_dt, kind="ExternalInput").ap()
    wkT = nc.dram_tensor("wkT", [D, DLOC], io_dt, kind="ExternalInput").ap()
    wvT = nc.dram_tensor("wvT", [D, DLOC], io_dt, kind="ExternalInput").ap()
    woutT = nc.dram_tensor("woutT", [DLOC, D], io_dt, kind="ExternalInput").ap()
    out = nc.dram_tensor("out", [N, D], f32, kind="ExternalOutput").ap()
    dbg_t = {}
    if dbg:
        DT2_ = DLOC // P
        for nm, shp in (
            ("dq", [P, DT2_, N]), ("dk", [P, DT2_, N]),
            ("dv", [P, N // P, HLOC, DH + 1]), ("do", [P, DT2_, N]),
        ):
            dbg_t[nm] = nc.dram_tensor(nm, shp, io_dt, kind="ExternalOutput").ap()

    CT = D // P  # 8 contraction tiles for projections
    NT = N // P  # 16 seq tiles
    DT2 = DLOC // P  # 2 local d-blocks (2 heads each)

    with tile.TileContext(nc) as tc:
        with (
            tc.tile_pool(name="big", bufs=1) as big,
            tc.tile_pool(name="stage", bufs=3) as stage,
            tc.tile_pool(name="obst", bufs=2) as obst,
            tc.tile_pool(name="small", bufs=1) as small,
            tc.tile_pool(name="ps_s", bufs=2, space="PSUM") as ps_s,
            tc.tile_pool(name="ps_o", bufs=2, space="PSUM") as ps_o,
            tc.tile_pool(name="ps_op", bufs=2, space="PSUM") as ps_op,
        ):
            ones_sb = small.tile([1, DH], f32r, tag="ones")
            nc.vector._memset_packed(
                ones_sb[:].bitcast(mybir.dt.uint32), 0x3F800000
            )

            for rep in range(repeat):
                _emit_iter(
                    nc, tile, mybir, f32, f32r, io_dt, rep, variants,
                    big, stage, obst, small, ps_s, ps_o, ps_op, ones_sb,
                    xT, wqT, wkT, wvT, woutT, out,
                    CT, NT, DT2, dbg_t,
                )

    nc.compile()
    return nc


def _emit_iter(
    nc, tile, mybir, f32, f32r, io_dt, rep, variants,
    big, stage, obst, small, ps_s, ps_o, ps_op, ones_sb,
    xT, wqT, wkT, wvT, woutT, out,
    CT, NT, DT2, dbg_t=None,
):
    Exp = mybir.ActivationFunctionType.Exp

    # ---- resident SBUF tensors ------------------------------------
    xT_sb = big.tile([P, CT, N], io_dt, tag="xT", name=f"xT_sb_{rep}")
    wqT_sb = big.tile([P, CT, DLOC], io_dt, tag="wqT", name=f"wqT_sb_{rep}")
    wkT_sb = big.tile([P, CT, DLOC], io_dt, tag="wkT", name=f"wkT_sb_{rep}")
    wvT_sb = big.tile([P, CT, DLOC], io_dt, tag="wvT", name=f"wvT_sb_{rep}")
    woutT_sb = big.tile([P, DT2, D], io_dt, tag="woutT", name=f"woutT_sb_{rep}")
    qT_sb = big.tile([P, DT2, N], io_dt, tag="qT", name=f"qT_sb_{rep}")
    kT_sb = big.tile([P, DT2, N], io_dt, tag="kT", name=f"kT_sb_{rep}")
    v_sb = big.tile([P, NT, HLOC, DH + 1], io_dt, tag="v", name=f"v_sb_{rep}")
    oT_sb = big.tile([P, DT2, N], io_dt, tag="oT", name=f"oT_sb_{rep}")
    # [ib%2] slot: a head's bc filler may pop up to one full i-block after
    # its reciprocal was produced, while the next i-block's reciprocal for
    # the same head is already being written.
    rec_sb = small.tile([1, 2, HLOC, IB], f32r, tag="rec", name=f"rec_{rep}")

    # ones column of v_aug (softmax denominator accumulates in row 64 of
    # the o PSUM).  memset can't emit bf16/f32r, so write bit patterns.
    col = v_sb[:, :, :, DH]
    if io_dt == mybir.dt.bfloat16:
        nc.vector._memset_packed(col.bitcast(mybir.dt.uint16), 0x3F80)
    elif io_dt == mybir.dt.float32r:
        nc.vector._memset_packed(col.bitcast(mybir.dt.uint32), 0x3F800000)
    else:
        nc.vector.memset(col, 1.0)

    # ---- input DMAs in need order ---------------------------------
    nc.sync.dma_start(wkT_sb[:], wkT.rearrange("(c p) d -> p c d", p=P))
    for ct in range(3):
        nc.sync.dma_start(xT_sb[:, ct, :], xT[ct * P : ct * P + P, :])
    nc.sync.dma_start(wqT_sb[:], wqT.rearrange("(c p) d -> p c d", p=P))
    for ct in range(3, CT):
        nc.sync.dma_start(xT_sb[:, ct, :], xT[ct * P : ct * P + P, :])
    nc.sync.dma_start(wvT_sb[:], wvT.rearrange("(c p) d -> p c d", p=P))
    nc.sync.dma_start(woutT_sb[:], woutT.rearrange("(t p) d -> p t d", p=P))

    # ---- projections ----------------------------------------------
    def proj_chunk_mms(w_sb, dt_, i0, iw, ps):
        # ps[:, :iw] = sum_c W[c, dt-block] . xT[c, i0:i0+iw]
        for ct in range(CT):
            for h0 in range(0, iw, 512):
                hw_ = min(512, iw - h0)
                yield lambda ct=ct, h0=h0, hw_=hw_: nc.tensor.matmul(
                    ps[:, h0 : h0 + hw_],
                    w_sb[:, ct, dt_ * P : dt_ * P + P],
                    xT_sb[:, ct, i0 + h0 : i0 + h0 + hw_],
                    start=(ct == 0),
                    stop=(ct == CT - 1),
                )

    def proj_emit(w_sb, dst, dt_, i0, iw, key):
        ps = ps_s.tile([P, 1024], f32, tag="s", name=f"ps_{key}_{rep}")
        for mm in proj_chunk_mms(w_sb, dt_, i0, iw, ps):
            mm()
        nc.vector.tensor_copy(dst[:, dt_, i0 : i0 + iw], ps[:, :iw])

    # k projection, fully (attention needs all key positions)
    for ih in range(2):
        for dt_ in range(DT2):
            proj_emit(wkT_sb, kT_sb, dt_, ih * 1024, 1024, f"k{ih}{dt_}")

    # v projection (natural [j, d] layout, fanned into the 65-wide slots)
    for jt in range(NT):
        psv = ps_s.tile([P, 1024], f32, tag="s", name=f"psv_{rep}_{jt}")
        for ct in range(CT):
            nc.tensor.matmul(
                psv[:, :DLOC],
                xT_sb[:, ct, jt * P : jt * P + P],
                wvT_sb[:, ct, :],
                start=(ct == 0),
                stop=(ct == CT - 1),
            )
        nc.vector.tensor_copy(
            v_sb[:, jt, :, :DH],
            psv[:, :DLOC].rearrange("p (h u) -> p h u", u=DH),
        )

    # q projection for the first i-block only; the rest becomes filler.
    for dt_ in range(DT2):
        proj_emit(wqT_sb, qT_sb, dt_, 0, IB, f"q0{dt_}")

    # ---- filler machinery ------------------------------------------
    # Each filler is (pe_cost, closure).  Per spine slot we pop fillers
    # until one PE-costed op has been emitted (plus any free DVE/DMA ops).
    fillers = []

    def push_proj_filler(w_sb, dst, dt_, i0, key):
        # q-projection chunks run out of the ps_op ring ([P, 512] tiles,
        # shared with out-proj/norm fillers) so they never steal a slot
        # from the spine's s-tile double buffer.
        ps_box = {}

        for idx in range(CT):
            def step(idx=idx):
                if idx == 0:
                    ps_box["ps"] = ps_op.tile(
                        [P, 512], f32, tag="op", name=f"ps_{key}_{rep}"
                    )
                nc.tensor.matmul(
                    ps_box["ps"][:],
                    w_sb[:, idx, dt_ * P : dt_ * P + P],
                    xT_sb[:, idx, i0 : i0 + IB],
                    start=(idx == 0),
                    stop=(idx == CT - 1),
                )
            fillers.append((1, step))
        fillers.append(
            (0, lambda: nc.vector.tensor_copy(
                dst[:, dt_, i0 : i0 + IB], ps_box["ps"][:]))
        )

    # remaining q chunks, in the order the spine will need them
    for ib in range(1, NIB):
        for dt_ in range(DT2):
            push_proj_filler(wqT_sb, qT_sb, dt_, ib * IB, f"q{ib}{dt_}")

    def push_norm_filler(ib, h):
        hp, ho = h // 2, (h % 2) * DH
        i0 = ib * IB

        box = {}

        def bc_mm():
            bc = ps_op.tile([P, 512], f32, tag="op", name=f"bc_{rep}_{ib}_{h}")
            box["bc"] = bc
            nc.tensor.matmul(
                bc[:DH, :IB],
                ones_sb[:],
                rec_sb[:, ib % 2, h, :],
                start=True,
                stop=True,
            )

        def mul():
            dst = oT_sb[ho : ho + DH, hp, i0 : i0 + IB]
            nc.vector.tensor_mul(dst, dst, box["bc"][:DH, :IB])

        fillers.append((1, bc_mm))
        fillers.append((0, mul))

    def push_outproj_filler(ib):
        def push_one(it):
            ob_box = {}

            def alloc_ob():
                ob_box["ob"] = obst.tile(
                    [P, 1024], f32, tag="ob", name=f"ob_{rep}_{it}"
                )

            fillers.append((0, alloc_ob))

            def push_half(db):
                pp_box = {}

                def mk_mm(dt_):
                    def mm():
                        if dt_ == 0:
                            pp_box["pp"] = ps_op.tile(
                                [P, 512], f32, tag="op",
                                name=f"pso_{rep}_{it}_{db}",
                            )
                        nc.tensor.matmul(
                            pp_box["pp"][:],
                            oT_sb[:, dt_, it * P : it * P + P],
                            woutT_sb[:, dt_, db * 512 : db * 512 + 512],
                            start=(dt_ == 0),
                            stop=(dt_ == DT2 - 1),
                        )
                    return mm

                for dt_ in range(DT2):
                    fillers.append((1, mk_mm(dt_)))
                fillers.append(
                    (0, lambda: nc.vector.tensor_copy(
                        ob_box["ob"][:, db * 512 : db * 512 + 512],
                        pp_box["pp"][:]))
                )

            for db in range(2):
                push_half(db)
            fillers.append(
                (0, lambda: nc.sync.dma_start(
                    out[it * P : it * P + P, :], ob_box["ob"][:]))
            )

        for k in range(4):
            push_one(ib * 4 + k)

    def pop_fillers(budget=1):
        if "nofill" in variants:
            return
        spent = 0
        while fillers and spent < budget:
            cost, fn = fillers.pop(0)
            fn()
            spent += cost
        # trailing free ops ride along with the last costed op
        while fillers and fillers[0][0] == 0:
            fillers.pop(0)[1]()

    # ---- attention spine -------------------------------------------
    noexp = "noexp" in variants
    for ib in range(NIB):
        i0 = ib * IB
        for hp in range(DT2):
            hA, hB = 2 * hp, 2 * hp + 1
            poA = ps_o.tile([DH + 1, IB], f32, tag="o", name=f"po_{rep}_{ib}_{hA}")
            poB = ps_o.tile([DH + 1, IB], f32, tag="o", name=f"po_{rep}_{ib}_{hB}")
            for jt in range(NT):
                pss = ps_s.tile(
                    [P, 1024], f32, tag="s", name=f"pss_{rep}_{ib}_{hp}_{jt}"
                )
                # the two dh=64 s-matmuls run concurrently on PE row
                # groups 0-63 / 64-127 (implicit tile_position row tiling)
                nc.tensor.matmul(
                    pss[:, 0:512],
                    kT_sb[0:DH, hp, jt * P : jt * P + P],
                    qT_sb[0:DH, hp, i0 : i0 + IB],
                    start=True, stop=True,
                )
                nc.tensor.matmul(
                    pss[:, 512:1024],
                    kT_sb[DH:P, hp, jt * P : jt * P + P],
                    qT_sb[DH:P, hp, i0 : i0 + IB],
                    start=True, stop=True,
                )
                p_sb = stage.tile(
                    [P, 1024], io_dt, tag="p", name=f"p_{rep}_{ib}_{hp}_{jt}"
                )
                if noexp:
                    nc.vector.tensor_copy(p_sb[:], pss[:])
                else:
                    nc.scalar.activation(p_sb[:], pss[:], Exp, scale=SCALE)
                nc.tensor.matmul(
                    poA[:],
                    v_sb[:, jt, hA, :],
                    p_sb[:, 0:512],
                    start=(jt == 0), stop=(jt == NT - 1),
                )
                nc.tensor.matmul(
                    poB[:],
                    v_sb[:, jt, hB, :],
                    p_sb[:, 512:1024],
                    start=(jt == 0), stop=(jt == NT - 1),
                )
                pop_fillers(1)
            # epilogue: reciprocal of the denominator row, drain the
            # unnormalized oT; normalization itself is deferred filler.
            for h, po in ((hA, poA), (hB, poB)):
                ho = (h % 2) * DH
                # f32r output is bit-identical fp32; the flag only exists
                # so the K=1 broadcast matmul runs at 1 cycle/row.
                with nc.allow_low_precision(reason="f32r recip is full fp32"):
                    nc.vector.reciprocal(
                        rec_sb[:, ib % 2, h, :], po[DH : DH + 1, :]
                    )
                nc.vector.tensor_copy(
                    oT_sb[ho : ho + DH, hp, i0 : i0 + IB], po[:DH, :]
                )
                push_norm_filler(ib, h)
        push_outproj_filler(ib)

    # drain whatever filler work is left (last i-block's norm + out-proj)
    while fillers:
        fillers.pop(0)[1]()

    if dbg_t:
        nc.sync.dma_start(dbg_t["dq"][:], qT_sb[:])
        nc.sync.dma_start(dbg_t["dk"][:], kT_sb[:])
        nc.sync.dma_start(dbg_t["dv"][:], v_sb[:])
        nc.sync.dma_start(dbg_t["do"][:], oT_sb[:])


def get_nc(mm_mode=MM_MODE, repeat=1):
    key = (mm_mode, repeat)
    if key not in _cached:
        _cached[key] = _build(mm_mode, repeat)
    return _cached[key]


def make_in_maps(x, Wq, Wk, Wv, Wout, mm_mode=MM_MODE):
    mm_mode = mm_mode.split("+")[0]
    if mm_mode == "bf16":
        import ml_dtypes

        cast = lambda a: np.ascontiguousarray(np.asarray(a), dtype=ml_dtypes.bfloat16)
    else:
        cast = lambda a: np.ascontiguousarray(np.asarray(a), dtype=np.float32)
    x, Wq, Wk, Wv, Wout = (np.asarray(a) for a in (x, Wq, Wk, Wv, Wout))
    in_maps = []
    for c in range(NCORES):
        b = c // 4
        rows = slice((c % 4) * DLOC, (c % 4 + 1) * DLOC)
        in_maps.append(
            {
                "xT": cast(x[b].T),
                "wqT": cast(Wq[rows].T),
                "wkT": cast(Wk[rows].T),
                "wvT": cast(Wv[rows].T),
                "woutT": cast(Wout[:, rows].T),
            }
        )
    return in_maps


def kernel(x, Wq, Wk, Wv, Wout, bout):
    from concourse.bass_utils import run_bass_kernel_spmd

    nc = get_nc()
    in_maps = make_in_maps(x, Wq, Wk, Wv, Wout)
    res = run_bass_kernel_spmd(nc, in_maps, list(range(NCORES)))
    out = np.zeros((B, N, D), np.float32)
    for c in range(NCORES):
        out[c // 4] += res.results[c]["out"]
    out += np.asarray(bout, np.float32)
    return out
